# revision 6
# baseline (speedup 1.0000x reference)
"""Depthwise causal conv1d (W=8) with 3 interleaved weight sets, on 8 TRN2 cores.

Reference computes r/o/a = depthwise_causal_conv(x, {rtg,obs,act}_{w,b}) and
interleaves out[:, t] = {r,o,a}[:, t] by t % 3.  Only the t%3-matching third of
each conv is needed, so total work is exactly one conv: for each output t,
out[b,t,h] = sum_k x[b, t-7+k, h] * w_{t%3}[h, k] + b_{t%3}[h].

Strategy (channel-parallel, 96 channels per core, banded-Toeplitz matmul):
  - time goes on the PE contraction axis: outputs are computed in blocks of
    ST=120 consecutive t per matmul column block.  For one channel, one
    [128 x 120] fp16 stationary matrix T with T[m, p] = w_{p%3}[ch, m-p]
    (0 <= m-p < 8) computes out[t0+p] = sum_k w[k] x[t0+p-7+k] for a whole
    block from a [128]-long input window x[t0-7 .. t0+120].  120 % 3 == 0
    keeps the phase pattern identical across blocks, so ONE stationary
    matrix per channel serves all (batch, block) pairs: rhs = [128 x 416]
    (416 = 16 batches x 26 blocks), i.e. 8 useful MACs per column-row vs 1
    for the diag formulation -- 8x fewer PE columns than the baseline.
  - row 127 of the window is unused by the band (max m = 119+7); the host
    stages a constant 1.0 there and T[127, p] = b_{p%3}[ch], folding the
    bias into the matmul.
  - PSUM f32 accumulates; ACT/DVE downcast-evict [120 x 4ch*416] to fp16;
    out-DMAs dispatched from the ACT HWDGE, in-DMAs from SP, so big
    contiguous descriptors spread over all 16 DMA engines (~23 GB/s each).
  - host pre/post stages the overlapped-window layout (fp16, unit-stride).
fp16 end-to-end rel err ~6e-4.
"""

import os
import numpy as np
from numpy.lib.stride_tricks import as_strided

B, T, H, W = 16, 3072, 768, 8
NCORES = 8
HC = H // NCORES             # 96 channels per core
ST = 120                     # outputs per block (multiple of 3)
NB = 26                      # blocks cover NB*ST = 3120 >= T
PADL = W - 1                 # causal left zero-pad
XLEN = ST * (NB - 1) + 128   # 3128 padded time extent (incl. window overhang)
COLS = B * NB                # 416 rhs columns per channel
CG = 4                       # channels per pipeline iteration (1 PSUM bank each)
NIT = HC // CG               # 24 iterations

_cache = {}


def _build_nc():
    import concourse.bacc as bacc
    import concourse.mybir as mybir
    import concourse.tile as tile

    nc = bacc.Bacc("TRN2", target_bir_lowering=False, debug=False)
    f32 = mybir.dt.float32
    f16 = mybir.dt.float16

    x_d = nc.dram_tensor("x", [NIT, 128, CG * COLS], f16, kind="ExternalInput").ap()
    w_d = nc.dram_tensor("w", [NIT, 128, CG * ST], f16, kind="ExternalInput").ap()
    y_d = nc.dram_tensor("y", [NIT, ST, CG * COLS], f16, kind="ExternalOutput").ap()

    with tile.TileContext(nc) as tc:
        with (
            tc.tile_pool(name="wp", bufs=3) as wp,
            tc.tile_pool(name="xp", bufs=4) as xp,
            tc.tile_pool(name="op", bufs=4) as op_,
            tc.tile_pool(name="ps", bufs=2, space="PSUM") as psp,
        ):
            for it in range(NIT):
                # per-iteration weight chunk keeps the fill short: the first
                # matmul only waits on one 120 KB + one 213 KB transfer
                wt = wp.tile([128, CG * ST], f16, tag="w")
                xt = xp.tile([128, CG * COLS], f16, tag="x")
                if it == 0:
                    # halve the first x transfer so matmul 0 starts sooner
                    nc.sync.dma_start(xt[:, : 2 * COLS], x_d[it][:, : 2 * COLS])
                    nc.sync.dma_start(wt[:], w_d[it])
                    nc.sync.dma_start(xt[:, 2 * COLS :], x_d[it][:, 2 * COLS :])
                else:
                    nc.sync.dma_start(wt[:], w_d[it])
                    nc.sync.dma_start(xt[:], x_d[it])
                ps = psp.tile([ST, CG, 512], f32, tag="ps")
                for c in range(CG):
                    nc.tensor.matmul(
                        ps[:, c, 0:COLS],
                        wt[:, c * ST : (c + 1) * ST],
                        xt[:, c * COLS : (c + 1) * COLS],
                        start=True, stop=True,
                    )
                ot = op_.tile([ST, CG, COLS], f16, tag="o")
                # split the PSUM->fp16 eviction ~3:2 DVE:ACT (ACT also
                # dispatches the out-DMAs); both are well under the DMA wall.
                # First/last iteration evict+store in 2ch halves to shorten
                # the pipeline fill and drain.
                if it == 0 or it == NIT - 1:
                    for h in range(2):
                        sl = slice(2 * h, 2 * h + 2)
                        if h == 0:
                            nc.vector.tensor_scalar_mul(ot[:, sl], ps[:, sl, 0:COLS], 1.0)
                        else:
                            nc.scalar.copy(ot[:, sl], ps[:, sl, 0:COLS])
                        nc.scalar.dma_start(
                            y_d[it][:, 2 * h * COLS : (2 * h + 2) * COLS], ot[:, sl]
                        )
                elif it % 5 < 3:
                    nc.vector.tensor_scalar_mul(ot[:], ps[:, :, 0:COLS], 1.0)
                    nc.scalar.dma_start(y_d[it], ot[:])
                else:
                    nc.scalar.copy(ot[:], ps[:, :, 0:COLS])
                    nc.scalar.dma_start(y_d[it], ot[:])

    nc.compile()
    return nc


def _get_nc():
    if "nc" not in _cache:
        _cache["nc"] = _build_nc()
    return _cache["nc"]


def _install_ntff_hook():
    """antenv.axon_hooks is not shipped in this container; shim it so
    bass_utils can find the NTFF profile hook (trace=True path)."""
    import sys, types
    if "antenv.axon_hooks" in sys.modules:
        return
    mod = types.ModuleType("antenv.axon_hooks")
    mod._hook = None
    mod.set_axon_ntff_profile_hook = lambda h: setattr(mod, "_hook", h)
    mod.get_axon_ntff_profile_hook = lambda: mod._hook
    sys.modules["antenv.axon_hooks"] = mod
    try:
        from trn_agent_boot.trn_boot import _ntff_profile_via_ctypes
        mod._hook = _ntff_profile_via_ctypes("/opt/axon/libaxon_pjrt.so")
    except Exception:
        mod._hook = None


def kernel(x, rtg_w, rtg_b, obs_w, obs_b, act_w, act_b):
    from concourse import bass_utils

    x = np.asarray(x, dtype=np.float32)
    ws = np.stack([np.asarray(a, np.float32) for a in (rtg_w, obs_w, act_w)], 1)  # [H,3,W]
    bs = np.stack([np.asarray(a, np.float32) for a in (rtg_b, obs_b, act_b)], 1)  # [H,3]

    # staged input windows: xs[ch, m, (b, n)] = x[b, ST*n + m - PADL, ch]
    xT = np.ascontiguousarray(x.transpose(2, 0, 1)).astype(np.float16)  # [H,B,T]
    xpad = np.zeros((H, B, XLEN), np.float16)
    xpad[:, :, PADL : PADL + T] = xT
    s = xpad.strides
    xs = as_strided(xpad, (H, B, NB, 128), (s[0], s[1], ST * s[2], s[2]))
    xs = np.ascontiguousarray(xs.transpose(0, 3, 1, 2)).reshape(H, 128, COLS)
    xs[:, 127, :] = 1.0  # feeds the bias row of the stationary matrix

    # stationary matrices: lh[ch, m, p] = w_{p%3}[ch, m-p], row 127 = bias
    lh = np.zeros((H, 128, ST), np.float32)
    pidx = np.arange(ST)
    for k in range(W):
        lh[:, pidx + k, pidx] = ws[:, pidx % 3, k]
    lh[:, 127, pidx] = bs[:, pidx % 3]
    lh = lh.astype(np.float16)

    in_maps = []
    for c in range(NCORES):
        ch0 = c * HC
        xc = xs[ch0 : ch0 + HC].reshape(NIT, CG, 128, COLS)
        xc = np.ascontiguousarray(xc.transpose(0, 2, 1, 3)).reshape(NIT, 128, CG * COLS)
        wc = lh[ch0 : ch0 + HC].reshape(NIT, CG, 128, ST)
        wc = np.ascontiguousarray(wc.transpose(0, 2, 1, 3)).reshape(NIT, 128, CG * ST)
        in_maps.append({"x": xc, "w": wc})

    nc = _get_nc()
    trace = bool(int(os.environ.get("KERNEL_TRACE", "0")))
    if trace:
        _install_ntff_hook()
    res = bass_utils.run_bass_kernel_spmd(
        nc, in_maps, core_ids=list(range(NCORES)), trace=trace,
    )
    _cache["last_result"] = res

    out = np.empty((B, T, H), dtype=np.float32)
    for c in range(NCORES):
        y = res.results[c]["y"]                                   # [NIT, ST, CG*COLS]
        y = y.reshape(NIT, ST, CG, B, NB).transpose(3, 4, 1, 0, 2)  # [B, NB, ST, NIT, CG]
        y = y.reshape(B, NB * ST, HC)[:, :T]
        out[:, :, c * HC : (c + 1) * HC] = y.astype(np.float32)
    return out
